# revision 15
# baseline (speedup 1.0000x reference)
"""Trainium2 Bass kernel for nn_FDN_88012469830490.

FDN reverb: IR synthesis (host, tiny: 6x6 solves) + FFT convolution
(device, heavy) of x (16,2,441000) with the 2x2x88200 IR.

Device algorithm per core (2 batches/core):
  overlap-save conv, FFT size N=262144 = 128*128*16, hop 173945, 3 blocks.
  Channel packing z = x0 + j*x1; y[b,o] = Re(conv(z_b, h[o,0]-j*h[o,1])).
  FFT = 3-stage matmul Cooley-Tukey on TensorE (PE transposes between
  stages), twiddle cmuls on DVE from PSUM, copies on ACT, spectral muls
  split DVE/GpSimd.
"""
import sys
import numpy as np

sys.path.insert(0, "/opt/trn_rl_repo")

# ---------------- problem constants ----------------
SR = 44100
DELAYS = np.array([997, 1153, 1327, 1559, 1801, 2099])
ND = 6
L = 88200
FB = L // 2 + 1
NDF = 49
T60 = 1.5
GAMMA_MAX = 10.0 ** ((-60.0 / SR / T60 * DELAYS) / 20.0)

T = 441000
N = 262144            # FFT size = 128*128*16
P1, P2, P3 = 128, 128, 16
M2 = P2 * P3          # 2048
HOP = N - (L - 1)     # 173945
NBLK = 3
NCORES = 8
USE_F32R = False      # f32r needs producer-side rounding (BIR verifier); fp32 for now

# valid overlap-save output region: rows 43 (tail from col 135) .. 127
VROW, VCOL = 43, 135  # L-1 = 88199 = 43*2048 + 135
ROW_TAIL = 2048 - VCOL            # 1913
FULL_OUT = (P1 - VROW - 1) * M2   # 84 rows * 2048 = 172032; 1913+172032 = HOP


# ---------------- host IR synthesis ----------------
def _expm_skew(S):
    """expm of a real skew-symmetric matrix via Hermitian eigendecomposition."""
    lam, V = np.linalg.eigh(1j * S)      # 1j*S is Hermitian
    return (V @ np.diag(np.exp(-1j * lam)) @ V.conj().T).real


def _host_ir(b, c, U_raw, gamma_raw):
    tri = np.triu(U_raw.astype(np.float64), 1)
    U = _expm_skew(tri - tri.T)
    gamma = (1.0 / (1.0 + np.exp(-gamma_raw.astype(np.float64)))) * GAMMA_MAX
    pos = np.arange(FB) * ((NDF - 1) / (FB - 1))
    i0 = np.clip(np.floor(pos).astype(int), 0, NDF - 2)
    frac = (pos - i0)[:, None]
    g = gamma[i0] * (1 - frac) + gamma[i0 + 1] * frac
    A = U[None, :, :] * g[:, None, :]
    freqs = np.arange(FB) / L * 2 * np.pi
    invD = np.exp(1j * freqs[:, None] * DELAYS)
    Mm = invD[:, :, None] * np.eye(ND) - A
    bc = np.broadcast_to(b.astype(np.float64), (FB, ND, 2))
    X = np.linalg.solve(Mm, bc)
    H = np.einsum('ci,fio->fco', c.astype(complex), X)      # (FB, 2, 2)
    h = np.fft.irfft(H.transpose(1, 2, 0), n=L)             # (2, 2, L)
    # w_o = h[o,0] - j*h[o,1], zero-padded to N, in (128, 2048) row layout
    wt = np.zeros((2, 2, P1, M2), np.float32)
    for o in range(2):
        wr = np.zeros(N, np.float64); wr[:L] = h[o, 0]
        wi = np.zeros(N, np.float64); wi[:L] = -h[o, 1]
        wt[o, 0] = wr.reshape(P1, M2).astype(np.float32)
        wt[o, 1] = wi.reshape(P1, M2).astype(np.float32)
    return wt


# ---------------- device-side constants ----------------
def _consts():
    F = np.exp(-2j * np.pi * np.outer(np.arange(P1), np.arange(P1)) / P1)
    F16 = np.exp(-2j * np.pi * np.outer(np.arange(P3), np.arange(P3)) / P3)
    B = np.zeros((128, 128), complex)
    for k in range(8):
        B[k*16:(k+1)*16, k*16:(k+1)*16] = F16
    fmat = np.stack([F.real, F.imag, -F.imag]).astype(np.float32)
    bmat = np.stack([B.real, B.imag, -B.imag]).astype(np.float32)
    ident = np.eye(128, dtype=np.float32)
    # t1[k1, m] = W_N^{k1*m}
    t1c = np.exp(-2j * np.pi * np.outer(np.arange(P1), np.arange(M2)) / N)
    # t2[k2, k1*16+n3] = W_{2048}^{k2*n3}
    n3v = np.tile(np.arange(P3), P2)
    t2c = np.exp(-2j * np.pi * np.outer(np.arange(P2), n3v) / M2)
    # it1[n2, k1*16+n3] = W_N^{k1*(n2*16+n3)}  (conj applied in cmul)
    k1v = np.repeat(np.arange(P1), P3)[None, :]
    n3b = np.tile(np.arange(P3), P2)[None, :]
    n2v = np.arange(P2)[:, None]
    it1c = np.exp(-2j * np.pi * (k1v * (n2v * P3 + n3b)) / N)
    t1 = np.stack([t1c.real, t1c.imag]).astype(np.float32)
    t2 = np.stack([t2c.real, t2c.imag]).astype(np.float32)
    it1 = np.stack([it1c.real, it1c.imag]).astype(np.float32)
    return fmat, bmat, ident, t1, t2, it1


# ---------------- bass program ----------------
_PROG = None


def _mmdt(ap):
    from concourse import mybir
    return ap.bitcast(mybir.dt.float32r) if USE_F32R else ap


def _build_program():
    import concourse.bass as bass
    import concourse.tile as tile
    from concourse import bacc, mybir

    f32 = mybir.dt.float32
    nc = bacc.Bacc("TRN2", target_bir_lowering=False, debug=False,
                   enable_asserts=False, num_devices=NCORES)

    xp = nc.dram_tensor("xp", [2, 2, T], f32, kind="ExternalInput").ap()
    wt = nc.dram_tensor("wt", [2, 2, P1, M2], f32, kind="ExternalInput").ap()
    fmat_d = nc.dram_tensor("fmat", [3, 128, 128], f32, kind="ExternalInput").ap()
    bmat_d = nc.dram_tensor("bmat", [3, 128, 128], f32, kind="ExternalInput").ap()
    id_d = nc.dram_tensor("ident", [128, 128], f32, kind="ExternalInput").ap()
    t1_d = nc.dram_tensor("t1", [2, 128, M2], f32, kind="ExternalInput").ap()
    t2_d = nc.dram_tensor("t2", [2, 128, M2], f32, kind="ExternalInput").ap()
    it1_d = nc.dram_tensor("it1", [2, 128, M2], f32, kind="ExternalInput").ap()
    yp = nc.dram_tensor("yp", [2, 2, T], f32, kind="ExternalOutput").ap()

    NCH = 4           # 512-wide chunks per 2048
    CW = 512

    from contextlib import ExitStack
    with tile.TileContext(nc) as tc, ExitStack() as ctx:
        cpool = ctx.enter_context(tc.tile_pool(name="consts", bufs=1))
        wpool = ctx.enter_context(tc.tile_pool(name="wspec", bufs=1))
        work = ctx.enter_context(tc.tile_pool(name="work", bufs=1))
        psA = ctx.enter_context(tc.tile_pool(name="psA", bufs=2, space="PSUM"))
        psT = ctx.enter_context(tc.tile_pool(name="psT", bufs=2, space="PSUM"))
        psTc = ctx.enter_context(tc.tile_pool(name="psTc", bufs=1, space="PSUM"))

        # constant tiles
        fr = cpool.tile([128, 128], f32, tag="fr"); nc.sync.dma_start(fr[:], fmat_d[0])
        fi = cpool.tile([128, 128], f32, tag="fi"); nc.sync.dma_start(fi[:], fmat_d[1])
        nfi = cpool.tile([128, 128], f32, tag="nfi"); nc.sync.dma_start(nfi[:], fmat_d[2])
        br = cpool.tile([128, 128], f32, tag="br"); nc.sync.dma_start(br[:], bmat_d[0])
        bi = cpool.tile([128, 128], f32, tag="bi"); nc.sync.dma_start(bi[:], bmat_d[1])
        nbi = cpool.tile([128, 128], f32, tag="nbi"); nc.sync.dma_start(nbi[:], bmat_d[2])
        idt = cpool.tile([128, 128], f32, tag="idt"); nc.sync.dma_start(idt[:], id_d[:, :])
        t1r = cpool.tile([128, M2], f32, tag="t1r"); nc.sync.dma_start(t1r[:], t1_d[0])
        t1i = cpool.tile([128, M2], f32, tag="t1i"); nc.sync.dma_start(t1i[:], t1_d[1])
        t2r = cpool.tile([128, M2], f32, tag="t2r"); nc.sync.dma_start(t2r[:], t2_d[0])
        t2i = cpool.tile([128, M2], f32, tag="t2i"); nc.sync.dma_start(t2i[:], t2_d[1])
        it1r = cpool.tile([128, M2], f32, tag="it1r"); nc.sync.dma_start(it1r[:], it1_d[0])
        it1i = cpool.tile([128, M2], f32, tag="it1i"); nc.sync.dma_start(it1i[:], it1_d[1])

        wtiles = [[wpool.tile([128, M2], f32, tag=f"w{o}{p}", name=f"w{o}{p}")
                   for p in range(2)] for o in range(2)]

        def cmul_psum(dst_r, dst_i, pr, pi, trr, tri, conj, s0, w, eng1, eng2):
            """(dst_r+j dst_i)[:, s0:s0+w] = (pr + j pi) * (trr + j tri)[slice],
            psum inputs pr/pi (128, w). conj: multiply by conj of twiddle."""
            sc = work.tile([128, CW], f32, tag="cm1")
            sd = work.tile([128, CW], f32, tag="cm2")
            a = (s0, s0 + w)
            # psum-reading muls must be DVE; SBUF-only combines go to GpSimd
            eng1.tensor_mul(sc[:, :w], pr[:, :w], trr[:, a[0]:a[1]])
            eng1.tensor_mul(sd[:, :w], pi[:, :w], tri[:, a[0]:a[1]])
            if conj:
                eng2.tensor_add(dst_r[:, a[0]:a[1]], sc[:, :w], sd[:, :w])
            else:
                eng2.tensor_sub(dst_r[:, a[0]:a[1]], sc[:, :w], sd[:, :w])
            sc2 = work.tile([128, CW], f32, tag="cm3")
            sd2 = work.tile([128, CW], f32, tag="cm4")
            eng1.tensor_mul(sc2[:, :w], pr[:, :w], tri[:, a[0]:a[1]])
            eng1.tensor_mul(sd2[:, :w], pi[:, :w], trr[:, a[0]:a[1]])
            if conj:
                eng2.tensor_sub(dst_i[:, a[0]:a[1]], sd2[:, :w], sc2[:, :w])
            else:
                eng2.tensor_add(dst_i[:, a[0]:a[1]], sc2[:, :w], sd2[:, :w])

        def stage_mm(dst, rhs_r, rhs_i, mr, mi, nmi, conj, tw, post):
            """One FFT stage: out = (conj?)(F) @ (rhs_r + j rhs_i), then
            per-chunk post-processing. tw = (twr, twi, conj_tw) or None.
            dst = (dr, di) SBUF tiles; post='copy'/'cmul'/'real_scale'."""
            dr, di = dst
            for ch in range(NCH):
                s0 = ch * CW
                sl = (slice(None), slice(s0, s0 + CW))
                prr = psA.tile([128, CW], f32, tag="pr")
                pii = psA.tile([128, CW], f32, tag="pi")
                if not conj:
                    nc.tensor.matmul(prr[:], _mmdt(mr[:]), _mmdt(rhs_r[sl]), start=True, stop=False)
                    nc.tensor.matmul(prr[:], _mmdt(nmi[:]), _mmdt(rhs_i[sl]), start=False, stop=True)
                    nc.tensor.matmul(pii[:], _mmdt(mi[:]), _mmdt(rhs_r[sl]), start=True, stop=False)
                    nc.tensor.matmul(pii[:], _mmdt(mr[:]), _mmdt(rhs_i[sl]), start=False, stop=True)
                else:
                    nc.tensor.matmul(prr[:], _mmdt(mr[:]), _mmdt(rhs_r[sl]), start=True, stop=False)
                    nc.tensor.matmul(prr[:], _mmdt(mi[:]), _mmdt(rhs_i[sl]), start=False, stop=True)
                    nc.tensor.matmul(pii[:], _mmdt(mr[:]), _mmdt(rhs_i[sl]), start=True, stop=False)
                    nc.tensor.matmul(pii[:], _mmdt(nmi[:]), _mmdt(rhs_r[sl]), start=False, stop=True)
                if post == "cmul":
                    twr, twi, ctw = tw
                    cmul_psum(dr, di, prr, pii, twr, twi, ctw, s0, CW,
                              nc.vector, nc.gpsimd)
                elif post == "copy":
                    nc.scalar.copy(dr[sl], prr[:])
                    nc.scalar.copy(di[sl], pii[:])

        def stage_mm_real(dst_r, rhs_r, rhs_i, mr, mi, scale):
            """Last inverse stage: real part only: Fr@vr + Fi@vi, scaled."""
            for ch in range(NCH):
                s0 = ch * CW
                sl = (slice(None), slice(s0, s0 + CW))
                prr = psA.tile([128, CW], f32, tag="pr")
                nc.tensor.matmul(prr[:], _mmdt(mr[:]), _mmdt(rhs_r[sl]), start=True, stop=False)
                nc.tensor.matmul(prr[:], _mmdt(mi[:]), _mmdt(rhs_i[sl]), start=False, stop=True)
                nc.scalar.mul(dst_r[sl], prr[:], scale)

        def t_slices_strided(src_r, src_i, dst_r, dst_i):
            """T1/T1': (p; a*16+n3) -> (a; p*16+n3): 16 strided slice transposes
            per plane, batched 4 slices -> one (128,512) psum, 3D copy out."""
            for pl, (s, d) in enumerate(((src_r, dst_r), (src_i, dst_i))):
                for grp in range(4):
                    pt = psT.tile([128, CW], f32, tag="pt")
                    for q in range(4):
                        n3 = grp * 4 + q
                        nc.tensor.transpose(pt[:, q*128:(q+1)*128],
                                            s[:, n3:M2:16], idt[:])
                    # psum free = (q, a): copy to dst[:, a*16 + (grp*4+q)]
                    psrc = pt[:].rearrange("p (q a) -> p a q", q=4)
                    ddst = d[:].rearrange("p (a b) -> p a b", b=16)[:, :, grp*4:grp*4+4]
                    nc.scalar.copy(ddst, psrc)

        def t_slices_contig(src_r, src_i, dst_r, dst_i, tw=None):
            """T2/T2': 16 contiguous 128-wide transposes; batched by 4 into
            (128,512) psum; copy (or conj-twiddle-cmul) to same free offsets."""
            for grp in range(4):
                ptr = psTc.tile([128, CW], f32, tag="ptr")
                pti = psTc.tile([128, CW], f32, tag="pti")
                for q in range(4):
                    g = grp * 4 + q
                    nc.tensor.transpose(ptr[:, q*128:(q+1)*128],
                                        src_r[:, g*128:(g+1)*128], idt[:])
                    nc.tensor.transpose(pti[:, q*128:(q+1)*128],
                                        src_i[:, g*128:(g+1)*128], idt[:])
                s0 = grp * CW
                if tw is None:
                    nc.scalar.copy(dst_r[:, s0:s0+CW], ptr[:])
                    nc.scalar.copy(dst_i[:, s0:s0+CW], pti[:])
                else:
                    twr, twi, ctw = tw
                    cmul_psum(dst_r, dst_i, ptr, pti, twr, twi, ctw, s0, CW,
                              nc.vector, nc.gpsimd)

        def fwd_fft(in_r, in_i, zr, zi, ar, ai, br_t, bi_t):
            stage_mm((ar, ai), in_r, in_i, fr, fi, nfi, False,
                     (t1r, t1i, False), "cmul")
            t_slices_strided(ar, ai, br_t, bi_t)
            stage_mm((ar, ai), br_t, bi_t, fr, fi, nfi, False,
                     (t2r, t2i, False), "cmul")
            t_slices_contig(ar, ai, br_t, bi_t)
            stage_mm((zr, zi), br_t, bi_t, br, bi, nbi, False, None, "copy")

        def inv_fft(pr_t, pi_t, out_r, ar, ai, br_t, bi_t):
            stage_mm((ar, ai), pr_t, pi_t, br, bi, nbi, True, None, "copy")
            t_slices_contig(ar, ai, br_t, bi_t, tw=(t2r, t2i, True))
            stage_mm((ar, ai), br_t, bi_t, fr, fi, nfi, True,
                     (it1r, it1i, True), "cmul")
            t_slices_strided(ar, ai, br_t, bi_t)
            stage_mm_real(out_r, br_t, bi_t, fr, fi, 1.0 / N)

        def load_block(in_r, in_i, b, blk):
            for pl, t_ in ((0, in_r), (1, in_i)):
                src = xp[b, pl]
                dma = nc.sync if pl == 0 else nc.scalar
                if blk == 0:
                    nc.vector.memset(t_[0:VROW+1, :], 0.0)
                    dma.dma_start(t_[VROW:VROW+1, VCOL:M2], src[0:ROW_TAIL].rearrange('(a b) -> a b', a=1))
                    dma.dma_start(
                        t_[VROW+1:128, :],
                        src[ROW_TAIL:HOP].rearrange("(r m) -> r m", m=M2))
                elif blk == 1:
                    s0 = HOP - (L - 1)
                    dma.dma_start(
                        t_[:, :], src[s0:s0+N].rearrange("(r m) -> r m", m=M2))
                else:
                    s0 = 2 * HOP - (L - 1)
                    nfull = (T - s0) // M2            # 88 full rows
                    rem = (T - s0) - nfull * M2       # 1085
                    nc.vector.memset(t_[64:128, :], 0.0)  # 32-aligned start; DMA overwrites 64..88
                    dma.dma_start(
                        t_[0:nfull, :],
                        src[s0:s0+nfull*M2].rearrange("(r m) -> r m", m=M2))
                    dma.dma_start(t_[nfull:nfull+1, 0:rem], src[s0+nfull*M2:T].rearrange('(a b) -> a b', a=1))

        def store_block(out_r, b, o, blk):
            dst = yp[b, o]
            base = blk * HOP
            nc.scalar.dma_start(dst[base:base+ROW_TAIL].rearrange('(a b) -> a b', a=1), out_r[VROW:VROW+1, VCOL:M2])
            if blk < 2:
                nc.scalar.dma_start(
                    dst[base+ROW_TAIL:base+HOP].rearrange("(r m) -> r m", m=M2),
                    out_r[VROW+1:128, :])
            else:
                nrem = T - base - ROW_TAIL            # 91197
                nfull = nrem // M2                    # 44
                rem = nrem - nfull * M2               # 1085
                nc.scalar.dma_start(
                    dst[base+ROW_TAIL:base+ROW_TAIL+nfull*M2]
                        .rearrange("(r m) -> r m", m=M2),
                    out_r[VROW+1:VROW+1+nfull, :])
                nc.scalar.dma_start(dst[T-rem:T].rearrange('(a b) -> a b', a=1), out_r[VROW+1+nfull:VROW+2+nfull, 0:rem])

        def spectral_cmul(dr, di, zr_, zi_, wr_, wi_):
            for ch in range(4):
                s0 = ch * CW
                sl = (slice(None), slice(s0, s0 + CW))
                sc = work.tile([128, CW], f32, tag="sm1")
                sd = work.tile([128, CW], f32, tag="sm2")
                nc.vector.tensor_mul(sc[:, :], zr_[sl], wr_[sl])
                nc.gpsimd.tensor_mul(sd[:, :], zi_[sl], wi_[sl])
                nc.vector.tensor_sub(dr[sl], sc[:, :], sd[:, :])
                nc.vector.tensor_mul(sc[:, :], zr_[sl], wi_[sl])
                nc.gpsimd.tensor_mul(sd[:, :], zi_[sl], wr_[sl])
                nc.gpsimd.tensor_add(di[sl], sc[:, :], sd[:, :])

        # ---- program ----
        def wtile_pair():
            ar = work.tile([128, M2], f32, tag="ar", bufs=2)
            ai = work.tile([128, M2], f32, tag="ai", bufs=2)
            bt_r = work.tile([128, M2], f32, tag="btr", bufs=2)
            bt_i = work.tile([128, M2], f32, tag="bti", bufs=2)
            return ar, ai, bt_r, bt_i

        # W spectra (2 forward FFTs of host-padded w)
        for o in range(2):
            ar, ai, bt_r, bt_i = wtile_pair()
            nc.sync.dma_start(bt_r[:], wt[o, 0])
            nc.scalar.dma_start(bt_i[:], wt[o, 1])
            fwd_fft(bt_r, bt_i, wtiles[o][0], wtiles[o][1], ar, ai, bt_r, bt_i)

        for b in range(2):
            for blk in range(NBLK):
                zr = work.tile([128, M2], f32, tag="zr")
                zi = work.tile([128, M2], f32, tag="zi")
                ar, ai, bt_r, bt_i = wtile_pair()
                load_block(bt_r, bt_i, b, blk)
                fwd_fft(bt_r, bt_i, zr, zi, ar, ai, bt_r, bt_i)
                for o in range(2):
                    pr_t = work.tile([128, M2], f32, tag="pr_s")
                    pi_t = work.tile([128, M2], f32, tag="pi_s")
                    spectral_cmul(pr_t, pi_t, zr, zi,
                                  wtiles[o][0], wtiles[o][1])
                    ar2, ai2, bt_r2, bt_i2 = wtile_pair()
                    out_r = work.tile([128, M2], f32, tag="outr")
                    inv_fft(pr_t, pi_t, out_r, ar2, ai2, bt_r2, bt_i2)
                    store_block(out_r, b, o, blk)

    nc.compile()
    return nc


def _get_prog():
    global _PROG
    if _PROG is None:
        _PROG = _build_program()
    return _PROG


# ---------------- public entry ----------------
def kernel(x, b, c, U_raw, gamma_raw):
    from concourse import bass_utils

    x = np.ascontiguousarray(np.asarray(x, np.float32))
    wt = _host_ir(np.asarray(b, np.float32), np.asarray(c, np.float32),
                  np.asarray(U_raw, np.float32), np.asarray(gamma_raw, np.float32))
    fmat, bmat, ident, t1, t2, it1 = _consts()
    nc = _get_prog()

    in_maps = []
    for core in range(NCORES):
        in_maps.append({
            "xp": x[2*core:2*core+2],
            "wt": wt, "fmat": fmat, "bmat": bmat, "ident": ident,
            "t1": t1, "t2": t2, "it1": it1,
        })
    res = bass_utils.run_bass_kernel_spmd(nc, in_maps, core_ids=list(range(NCORES)))
    y = np.empty((16, 2, T), np.float32)
    for core in range(NCORES):
        y[2*core:2*core+2] = res.results[core]["yp"]
    return y


# revision 17
# speedup vs baseline: 1.0242x; 1.0242x over previous
"""Trainium2 Bass kernel for nn_FDN_88012469830490.

FDN reverb: IR synthesis (host, tiny: 6x6 solves) + FFT convolution
(device, heavy) of x (16,2,441000) with the 2x2x88200 IR.

Device algorithm per core (2 batches/core):
  overlap-save conv, FFT size N=262144 = 128*128*16, hop 173945, 3 blocks.
  Channel packing z = x0 + j*x1; y[b,o] = Re(conv(z_b, h[o,0]-j*h[o,1])).
  FFT = 3-stage matmul Cooley-Tukey on TensorE (PE transposes between
  stages), twiddle cmuls on DVE from PSUM, copies on ACT, spectral muls
  split DVE/GpSimd.
"""
import sys
import numpy as np

sys.path.insert(0, "/opt/trn_rl_repo")

# ---------------- problem constants ----------------
SR = 44100
DELAYS = np.array([997, 1153, 1327, 1559, 1801, 2099])
ND = 6
L = 88200
FB = L // 2 + 1
NDF = 49
T60 = 1.5
GAMMA_MAX = 10.0 ** ((-60.0 / SR / T60 * DELAYS) / 20.0)

T = 441000
N = 262144            # FFT size = 128*128*16
P1, P2, P3 = 128, 128, 16
M2 = P2 * P3          # 2048
HOP = N - (L - 1)     # 173945
NBLK = 3
NCORES = 8
USE_F32R = False      # f32r crashes the exec unit on TRN2 (NRT_EXEC_UNIT_UNRECOVERABLE); fp32 matmuls

# valid overlap-save output region: rows 43 (tail from col 135) .. 127
VROW, VCOL = 43, 135  # L-1 = 88199 = 43*2048 + 135
ROW_TAIL = 2048 - VCOL            # 1913
FULL_OUT = (P1 - VROW - 1) * M2   # 84 rows * 2048 = 172032; 1913+172032 = HOP


# ---------------- host IR synthesis ----------------
def _expm_skew(S):
    """expm of a real skew-symmetric matrix via Hermitian eigendecomposition."""
    lam, V = np.linalg.eigh(1j * S)      # 1j*S is Hermitian
    return (V @ np.diag(np.exp(-1j * lam)) @ V.conj().T).real


def _host_ir(b, c, U_raw, gamma_raw):
    tri = np.triu(U_raw.astype(np.float64), 1)
    U = _expm_skew(tri - tri.T)
    gamma = (1.0 / (1.0 + np.exp(-gamma_raw.astype(np.float64)))) * GAMMA_MAX
    pos = np.arange(FB) * ((NDF - 1) / (FB - 1))
    i0 = np.clip(np.floor(pos).astype(int), 0, NDF - 2)
    frac = (pos - i0)[:, None]
    g = gamma[i0] * (1 - frac) + gamma[i0 + 1] * frac
    A = U[None, :, :] * g[:, None, :]
    freqs = np.arange(FB) / L * 2 * np.pi
    invD = np.exp(1j * freqs[:, None] * DELAYS)
    Mm = invD[:, :, None] * np.eye(ND) - A
    bc = np.broadcast_to(b.astype(np.float64), (FB, ND, 2))
    X = np.linalg.solve(Mm, bc)
    H = np.einsum('ci,fio->fco', c.astype(complex), X)      # (FB, 2, 2)
    h = np.fft.irfft(H.transpose(1, 2, 0), n=L)             # (2, 2, L)
    # w_o = h[o,0] - j*h[o,1], zero-padded to N, in (128, 2048) row layout
    wt = np.zeros((2, 2, P1, M2), np.float32)
    for o in range(2):
        wr = np.zeros(N, np.float64); wr[:L] = h[o, 0]
        wi = np.zeros(N, np.float64); wi[:L] = -h[o, 1]
        wt[o, 0] = wr.reshape(P1, M2).astype(np.float32)
        wt[o, 1] = wi.reshape(P1, M2).astype(np.float32)
    return wt


# ---------------- device-side constants ----------------
def _consts():
    F = np.exp(-2j * np.pi * np.outer(np.arange(P1), np.arange(P1)) / P1)
    F16 = np.exp(-2j * np.pi * np.outer(np.arange(P3), np.arange(P3)) / P3)
    B = np.zeros((128, 128), complex)
    for k in range(8):
        B[k*16:(k+1)*16, k*16:(k+1)*16] = F16
    fmat = np.stack([F.real, F.imag, -F.imag]).astype(np.float32)
    bmat = np.stack([B.real, B.imag, -B.imag]).astype(np.float32)
    ident = np.eye(128, dtype=np.float32)
    # t1[k1, m] = W_N^{k1*m}
    t1c = np.exp(-2j * np.pi * np.outer(np.arange(P1), np.arange(M2)) / N)
    # t2[k2, k1*16+n3] = W_{2048}^{k2*n3}
    n3v = np.tile(np.arange(P3), P2)
    t2c = np.exp(-2j * np.pi * np.outer(np.arange(P2), n3v) / M2)
    # it1[n2, k1*16+n3] = W_N^{k1*(n2*16+n3)}  (conj applied in cmul)
    k1v = np.repeat(np.arange(P1), P3)[None, :]
    n3b = np.tile(np.arange(P3), P2)[None, :]
    n2v = np.arange(P2)[:, None]
    it1c = np.exp(-2j * np.pi * (k1v * (n2v * P3 + n3b)) / N)
    t1 = np.stack([t1c.real, t1c.imag]).astype(np.float32)
    t2 = np.stack([t2c.real, t2c.imag]).astype(np.float32)
    it1 = np.stack([it1c.real, it1c.imag]).astype(np.float32)
    return fmat, bmat, ident, t1, t2, it1


# ---------------- bass program ----------------
_PROG = None


def _mmdt(ap):
    return ap


def _build_program():
    import concourse.bass as bass
    import concourse.tile as tile
    from concourse import bacc, mybir

    f32 = mybir.dt.float32
    fmm = mybir.dt.float32r if USE_F32R else f32
    nc = bacc.Bacc("TRN2", target_bir_lowering=False, debug=False,
                   enable_asserts=False, num_devices=NCORES)

    xp = nc.dram_tensor("xp", [2, 2, T], f32, kind="ExternalInput").ap()
    wt = nc.dram_tensor("wt", [2, 2, P1, M2], f32, kind="ExternalInput").ap()
    fmat_d = nc.dram_tensor("fmat", [3, 128, 128], f32, kind="ExternalInput").ap()
    bmat_d = nc.dram_tensor("bmat", [3, 128, 128], f32, kind="ExternalInput").ap()
    id_d = nc.dram_tensor("ident", [128, 128], f32, kind="ExternalInput").ap()
    t1_d = nc.dram_tensor("t1", [2, 128, M2], f32, kind="ExternalInput").ap()
    t2_d = nc.dram_tensor("t2", [2, 128, M2], f32, kind="ExternalInput").ap()
    it1_d = nc.dram_tensor("it1", [2, 128, M2], f32, kind="ExternalInput").ap()
    yp = nc.dram_tensor("yp", [2, 2, T], f32, kind="ExternalOutput").ap()

    NCH = 4           # 512-wide chunks per 2048
    CW = 512

    from contextlib import ExitStack
    with tile.TileContext(nc) as tc, ExitStack() as ctx:
        cpool = ctx.enter_context(tc.tile_pool(name="consts", bufs=1))
        wpool = ctx.enter_context(tc.tile_pool(name="wspec", bufs=1))
        work = ctx.enter_context(tc.tile_pool(name="work", bufs=1))
        psA = ctx.enter_context(tc.tile_pool(name="psA", bufs=2, space="PSUM"))
        psT = ctx.enter_context(tc.tile_pool(name="psT", bufs=2, space="PSUM"))
        psTc = ctx.enter_context(tc.tile_pool(name="psTc", bufs=1, space="PSUM"))

        # constant tiles
        fr = cpool.tile([128, 128], f32, tag="fr"); nc.sync.dma_start(fr[:], fmat_d[0])
        fi = cpool.tile([128, 128], f32, tag="fi"); nc.sync.dma_start(fi[:], fmat_d[1])
        nfi = cpool.tile([128, 128], f32, tag="nfi"); nc.sync.dma_start(nfi[:], fmat_d[2])
        br = cpool.tile([128, 128], f32, tag="br"); nc.sync.dma_start(br[:], bmat_d[0])
        bi = cpool.tile([128, 128], f32, tag="bi"); nc.sync.dma_start(bi[:], bmat_d[1])
        nbi = cpool.tile([128, 128], f32, tag="nbi"); nc.sync.dma_start(nbi[:], bmat_d[2])
        idt = cpool.tile([128, 128], f32, tag="idt"); nc.sync.dma_start(idt[:], id_d[:, :])
        t1r = cpool.tile([128, M2], f32, tag="t1r"); nc.sync.dma_start(t1r[:], t1_d[0])
        t1i = cpool.tile([128, M2], f32, tag="t1i"); nc.sync.dma_start(t1i[:], t1_d[1])
        t2r = cpool.tile([128, M2], f32, tag="t2r"); nc.sync.dma_start(t2r[:], t2_d[0])
        t2i = cpool.tile([128, M2], f32, tag="t2i"); nc.sync.dma_start(t2i[:], t2_d[1])
        it1r = cpool.tile([128, M2], f32, tag="it1r"); nc.sync.dma_start(it1r[:], it1_d[0])
        it1i = cpool.tile([128, M2], f32, tag="it1i"); nc.sync.dma_start(it1i[:], it1_d[1])

        wtiles = [[wpool.tile([128, M2], f32, tag=f"w{o}{p}", name=f"w{o}{p}")
                   for p in range(2)] for o in range(2)]

        # device-rounded f32r copies of the DFT matrices (for compute-fed stages)
        fr_r = cpool.tile([128, 128], fmm, tag="fr_r"); nc.scalar.copy(fr_r[:], fr[:])
        fi_r = cpool.tile([128, 128], fmm, tag="fi_r"); nc.scalar.copy(fi_r[:], fi[:])
        nfi_r = cpool.tile([128, 128], fmm, tag="nfi_r"); nc.scalar.copy(nfi_r[:], nfi[:])
        br_r = cpool.tile([128, 128], fmm, tag="br_r"); nc.scalar.copy(br_r[:], br[:])
        bi_r = cpool.tile([128, 128], fmm, tag="bi_r"); nc.scalar.copy(bi_r[:], bi[:])
        nbi_r = cpool.tile([128, 128], fmm, tag="nbi_r"); nc.scalar.copy(nbi_r[:], nbi[:])

        def cmul_psum(dst_r, dst_i, pr, pi, trr, tri, conj, s0, w, eng1, eng2):
            """(dst_r+j dst_i)[:, s0:s0+w] = (pr + j pi) * (trr + j tri)[slice],
            psum inputs pr/pi (128, w). conj: multiply by conj of twiddle."""
            sc = work.tile([128, CW], f32, tag="cm1")
            sd = work.tile([128, CW], f32, tag="cm2")
            a = (s0, s0 + w)
            # psum-reading muls must be DVE; SBUF-only combines go to GpSimd
            eng1.tensor_mul(sc[:, :w], pr[:, :w], trr[:, a[0]:a[1]])
            eng1.tensor_mul(sd[:, :w], pi[:, :w], tri[:, a[0]:a[1]])
            if conj:
                eng2.tensor_add(dst_r[:, a[0]:a[1]], sc[:, :w], sd[:, :w])
            else:
                eng2.tensor_sub(dst_r[:, a[0]:a[1]], sc[:, :w], sd[:, :w])
            sc2 = work.tile([128, CW], f32, tag="cm3")
            sd2 = work.tile([128, CW], f32, tag="cm4")
            eng1.tensor_mul(sc2[:, :w], pr[:, :w], tri[:, a[0]:a[1]])
            eng1.tensor_mul(sd2[:, :w], pi[:, :w], trr[:, a[0]:a[1]])
            if conj:
                eng2.tensor_sub(dst_i[:, a[0]:a[1]], sd2[:, :w], sc2[:, :w])
            else:
                eng2.tensor_add(dst_i[:, a[0]:a[1]], sc2[:, :w], sd2[:, :w])

        def stage_mm(dst, rhs_r, rhs_i, mr, mi, nmi, conj, tw, post):
            """One FFT stage: out = (conj?)(F) @ (rhs_r + j rhs_i), then
            per-chunk post-processing. tw = (twr, twi, conj_tw) or None.
            dst = (dr, di) SBUF tiles; post='copy'/'cmul'/'real_scale'."""
            dr, di = dst
            for ch in range(NCH):
                s0 = ch * CW
                sl = (slice(None), slice(s0, s0 + CW))
                prr = psA.tile([128, CW], f32, tag="pr")
                pii = psA.tile([128, CW], f32, tag="pi")
                if not conj:
                    nc.tensor.matmul(prr[:], _mmdt(mr[:]), _mmdt(rhs_r[sl]), start=True, stop=False)
                    nc.tensor.matmul(prr[:], _mmdt(nmi[:]), _mmdt(rhs_i[sl]), start=False, stop=True)
                    nc.tensor.matmul(pii[:], _mmdt(mi[:]), _mmdt(rhs_r[sl]), start=True, stop=False)
                    nc.tensor.matmul(pii[:], _mmdt(mr[:]), _mmdt(rhs_i[sl]), start=False, stop=True)
                else:
                    nc.tensor.matmul(prr[:], _mmdt(mr[:]), _mmdt(rhs_r[sl]), start=True, stop=False)
                    nc.tensor.matmul(prr[:], _mmdt(mi[:]), _mmdt(rhs_i[sl]), start=False, stop=True)
                    nc.tensor.matmul(pii[:], _mmdt(mr[:]), _mmdt(rhs_i[sl]), start=True, stop=False)
                    nc.tensor.matmul(pii[:], _mmdt(nmi[:]), _mmdt(rhs_r[sl]), start=False, stop=True)
                if post == "cmul":
                    twr, twi, ctw = tw
                    cmul_psum(dr, di, prr, pii, twr, twi, ctw, s0, CW,
                              nc.vector, nc.gpsimd)
                elif post == "copy":
                    nc.scalar.copy(dr[sl], prr[:])
                    nc.scalar.copy(di[sl], pii[:])

        def stage_mm_real(dst_r, rhs_r, rhs_i, mr, mi, scale):
            """Last inverse stage: real part only: Fr@vr + Fi@vi, scaled."""
            for ch in range(NCH):
                s0 = ch * CW
                sl = (slice(None), slice(s0, s0 + CW))
                prr = psA.tile([128, CW], f32, tag="pr")
                nc.tensor.matmul(prr[:], _mmdt(mr[:]), _mmdt(rhs_r[sl]), start=True, stop=False)
                nc.tensor.matmul(prr[:], _mmdt(mi[:]), _mmdt(rhs_i[sl]), start=False, stop=True)
                nc.scalar.mul(dst_r[sl], prr[:], scale)

        def t_slices_strided(src_r, src_i, dst_r, dst_i):
            """T1/T1': (p; a*16+n3) -> (a; p*16+n3): 16 strided slice transposes
            per plane, batched 4 slices -> one (128,512) psum, 3D copy out."""
            for pl, (s, d) in enumerate(((src_r, dst_r), (src_i, dst_i))):
                for grp in range(4):
                    pt = psT.tile([128, CW], f32, tag="pt")
                    for q in range(4):
                        n3 = grp * 4 + q
                        nc.tensor.transpose(pt[:, q*128:(q+1)*128],
                                            s[:, n3:M2:16], idt[:])
                    # psum free = (q, a): copy to dst[:, a*16 + (grp*4+q)]
                    psrc = pt[:].rearrange("p (q a) -> p a q", q=4)
                    ddst = d[:].rearrange("p (a b) -> p a b", b=16)[:, :, grp*4:grp*4+4]
                    nc.scalar.copy(ddst, psrc)

        def t_slices_contig(src_r, src_i, dst_r, dst_i, tw=None):
            """T2/T2': 16 contiguous 128-wide transposes; batched by 4 into
            (128,512) psum; copy (or conj-twiddle-cmul) to same free offsets."""
            for grp in range(4):
                ptr = psTc.tile([128, CW], f32, tag="ptr")
                pti = psTc.tile([128, CW], f32, tag="pti")
                for q in range(4):
                    g = grp * 4 + q
                    nc.tensor.transpose(ptr[:, q*128:(q+1)*128],
                                        src_r[:, g*128:(g+1)*128], idt[:])
                    nc.tensor.transpose(pti[:, q*128:(q+1)*128],
                                        src_i[:, g*128:(g+1)*128], idt[:])
                s0 = grp * CW
                if tw is None:
                    nc.scalar.copy(dst_r[:, s0:s0+CW], ptr[:])
                    nc.scalar.copy(dst_i[:, s0:s0+CW], pti[:])
                else:
                    twr, twi, ctw = tw
                    cmul_psum(dst_r, dst_i, ptr, pti, twr, twi, ctw, s0, CW,
                              nc.vector, nc.gpsimd)

        def fwd_fft(in_r, in_i, zr, zi, ar, ai, br_t, bi_t):
            stage_mm((ar, ai), in_r, in_i, fr, fi, nfi, False,
                     (t1r, t1i, False), "cmul")
            t_slices_strided(ar, ai, br_t, bi_t)
            stage_mm((ar, ai), br_t, bi_t, fr_r, fi_r, nfi_r, False,
                     (t2r, t2i, False), "cmul")
            t_slices_contig(ar, ai, br_t, bi_t)
            stage_mm((zr, zi), br_t, bi_t, br_r, bi_r, nbi_r, False, None, "copy")

        def inv_fft(pr_t, pi_t, out_r, ar, ai, br_t, bi_t):
            stage_mm((ar, ai), pr_t, pi_t, br_r, bi_r, nbi_r, True, None, "copy")
            t_slices_contig(ar, ai, br_t, bi_t, tw=(t2r, t2i, True))
            stage_mm((ar, ai), br_t, bi_t, fr_r, fi_r, nfi_r, True,
                     (it1r, it1i, True), "cmul")
            t_slices_strided(ar, ai, br_t, bi_t)
            stage_mm_real(out_r, br_t, bi_t, fr_r, fi_r, 1.0 / N)

        def load_block(in_r, in_i, b, blk):
            for pl, t_ in ((0, in_r), (1, in_i)):
                src = xp[b, pl]
                dma = nc.sync if pl == 0 else nc.scalar
                if blk == 0:
                    nc.vector.memset(t_[0:VROW+1, :], 0.0)
                    dma.dma_start(t_[VROW:VROW+1, VCOL:M2], src[0:ROW_TAIL].rearrange('(a b) -> a b', a=1))
                    dma.dma_start(
                        t_[VROW+1:128, :],
                        src[ROW_TAIL:HOP].rearrange("(r m) -> r m", m=M2))
                elif blk == 1:
                    s0 = HOP - (L - 1)
                    dma.dma_start(
                        t_[:, :], src[s0:s0+N].rearrange("(r m) -> r m", m=M2))
                else:
                    s0 = 2 * HOP - (L - 1)
                    nfull = (T - s0) // M2            # 88 full rows
                    rem = (T - s0) - nfull * M2       # 1085
                    nc.vector.memset(t_[64:128, :], 0.0)  # 32-aligned start; DMA overwrites 64..88
                    dma.dma_start(
                        t_[0:nfull, :],
                        src[s0:s0+nfull*M2].rearrange("(r m) -> r m", m=M2))
                    dma.dma_start(t_[nfull:nfull+1, 0:rem], src[s0+nfull*M2:T].rearrange('(a b) -> a b', a=1))

        def store_block(out_r, b, o, blk):
            dst = yp[b, o]
            base = blk * HOP
            nc.scalar.dma_start(dst[base:base+ROW_TAIL].rearrange('(a b) -> a b', a=1), out_r[VROW:VROW+1, VCOL:M2])
            if blk < 2:
                nc.scalar.dma_start(
                    dst[base+ROW_TAIL:base+HOP].rearrange("(r m) -> r m", m=M2),
                    out_r[VROW+1:128, :])
            else:
                nrem = T - base - ROW_TAIL            # 91197
                nfull = nrem // M2                    # 44
                rem = nrem - nfull * M2               # 1085
                nc.scalar.dma_start(
                    dst[base+ROW_TAIL:base+ROW_TAIL+nfull*M2]
                        .rearrange("(r m) -> r m", m=M2),
                    out_r[VROW+1:VROW+1+nfull, :])
                nc.scalar.dma_start(dst[T-rem:T].rearrange('(a b) -> a b', a=1), out_r[VROW+1+nfull:VROW+2+nfull, 0:rem])

        def spectral_cmul(dr, di, zr_, zi_, wr_, wi_):
            for ch in range(4):
                s0 = ch * CW
                sl = (slice(None), slice(s0, s0 + CW))
                sc = work.tile([128, CW], f32, tag="sm1")
                sd = work.tile([128, CW], f32, tag="sm2")
                nc.vector.tensor_mul(sc[:, :], zr_[sl], wr_[sl])
                nc.gpsimd.tensor_mul(sd[:, :], zi_[sl], wi_[sl])
                nc.vector.tensor_sub(dr[sl], sc[:, :], sd[:, :])
                nc.vector.tensor_mul(sc[:, :], zr_[sl], wi_[sl])
                nc.gpsimd.tensor_mul(sd[:, :], zi_[sl], wr_[sl])
                nc.gpsimd.tensor_add(di[sl], sc[:, :], sd[:, :])

        # ---- program ----
        def wtile_pair():
            ar = work.tile([128, M2], f32, tag="ar")
            ai = work.tile([128, M2], f32, tag="ai")
            bt_r = work.tile([128, M2], fmm, tag="btr", bufs=2)
            bt_i = work.tile([128, M2], fmm, tag="bti", bufs=2)
            return ar, ai, bt_r, bt_i

        # W spectra (2 forward FFTs of host-padded w)
        for o in range(2):
            ar, ai, bt_r, bt_i = wtile_pair()
            in_r = work.tile([128, M2], f32, tag="inr")
            in_i = work.tile([128, M2], f32, tag="ini")
            nc.sync.dma_start(in_r[:], wt[o, 0])
            nc.scalar.dma_start(in_i[:], wt[o, 1])
            fwd_fft(in_r, in_i, wtiles[o][0], wtiles[o][1], ar, ai, bt_r, bt_i)

        for b in range(2):
            for blk in range(NBLK):
                zr = work.tile([128, M2], f32, tag="zr")
                zi = work.tile([128, M2], f32, tag="zi")
                ar, ai, bt_r, bt_i = wtile_pair()
                in_r = work.tile([128, M2], f32, tag="inr")
                in_i = work.tile([128, M2], f32, tag="ini")
                load_block(in_r, in_i, b, blk)
                fwd_fft(in_r, in_i, zr, zi, ar, ai, bt_r, bt_i)
                for o in range(2):
                    pr_t = work.tile([128, M2], fmm, tag="pr_s")
                    pi_t = work.tile([128, M2], fmm, tag="pi_s")
                    spectral_cmul(pr_t, pi_t, zr, zi,
                                  wtiles[o][0], wtiles[o][1])
                    ar2, ai2, bt_r2, bt_i2 = wtile_pair()
                    out_r = work.tile([128, M2], f32, tag="outr")
                    inv_fft(pr_t, pi_t, out_r, ar2, ai2, bt_r2, bt_i2)
                    store_block(out_r, b, o, blk)

    nc.compile()
    return nc


def _get_prog():
    global _PROG
    if _PROG is None:
        _PROG = _build_program()
    return _PROG


# ---------------- public entry ----------------
def kernel(x, b, c, U_raw, gamma_raw):
    from concourse import bass_utils

    x = np.ascontiguousarray(np.asarray(x, np.float32))
    wt = _host_ir(np.asarray(b, np.float32), np.asarray(c, np.float32),
                  np.asarray(U_raw, np.float32), np.asarray(gamma_raw, np.float32))
    fmat, bmat, ident, t1, t2, it1 = _consts()
    nc = _get_prog()

    in_maps = []
    for core in range(NCORES):
        in_maps.append({
            "xp": x[2*core:2*core+2],
            "wt": wt, "fmat": fmat, "bmat": bmat, "ident": ident,
            "t1": t1, "t2": t2, "it1": it1,
        })
    res = bass_utils.run_bass_kernel_spmd(nc, in_maps, core_ids=list(range(NCORES)))
    y = np.empty((16, 2, T), np.float32)
    for core in range(NCORES):
        y[2*core:2*core+2] = res.results[core]["yp"]
    return y


# revision 18
# speedup vs baseline: 2860.2553x; 2792.6236x over previous
"""Trainium2 Bass kernel for nn_FDN_88012469830490.

FDN reverb: IR synthesis (host, tiny: 6x6 solves) + FFT convolution
(device, heavy) of x (16,2,441000) with the 2x2x88200 IR.

Device algorithm per core (2 batches/core):
  overlap-save conv, FFT size N=262144 = 128*128*16, hop 173945, 3 blocks.
  Channel packing z = x0 + j*x1; y[b,o] = Re(conv(z_b, h[o,0]-j*h[o,1])).
  FFT = 3-stage matmul Cooley-Tukey on TensorE (PE transposes between
  stages), twiddle cmuls on DVE from PSUM, copies on ACT, spectral muls
  split DVE/GpSimd.
"""
import sys
import numpy as np

sys.path.insert(0, "/opt/trn_rl_repo")

# ---------------- problem constants ----------------
SR = 44100
DELAYS = np.array([997, 1153, 1327, 1559, 1801, 2099])
ND = 6
L = 88200
FB = L // 2 + 1
NDF = 49
T60 = 1.5
GAMMA_MAX = 10.0 ** ((-60.0 / SR / T60 * DELAYS) / 20.0)

T = 441000
N = 262144            # FFT size = 128*128*16
P1, P2, P3 = 128, 128, 16
M2 = P2 * P3          # 2048
HOP = N - (L - 1)     # 173945
NBLK = 3
NCORES = 8
USE_F32R = False      # f32r crashes the exec unit on TRN2 (NRT_EXEC_UNIT_UNRECOVERABLE); fp32 matmuls

# valid overlap-save output region: rows 43 (tail from col 135) .. 127
VROW, VCOL = 43, 135  # L-1 = 88199 = 43*2048 + 135
ROW_TAIL = 2048 - VCOL            # 1913
FULL_OUT = (P1 - VROW - 1) * M2   # 84 rows * 2048 = 172032; 1913+172032 = HOP


# ---------------- host IR synthesis ----------------
def _expm_skew(S):
    """expm of a real skew-symmetric matrix via Hermitian eigendecomposition."""
    lam, V = np.linalg.eigh(1j * S)      # 1j*S is Hermitian
    return (V @ np.diag(np.exp(-1j * lam)) @ V.conj().T).real


def _host_ir(b, c, U_raw, gamma_raw):
    tri = np.triu(U_raw.astype(np.float64), 1)
    U = _expm_skew(tri - tri.T)
    gamma = (1.0 / (1.0 + np.exp(-gamma_raw.astype(np.float64)))) * GAMMA_MAX
    pos = np.arange(FB) * ((NDF - 1) / (FB - 1))
    i0 = np.clip(np.floor(pos).astype(int), 0, NDF - 2)
    frac = (pos - i0)[:, None]
    g = gamma[i0] * (1 - frac) + gamma[i0 + 1] * frac
    A = U[None, :, :] * g[:, None, :]
    freqs = np.arange(FB) / L * 2 * np.pi
    invD = np.exp(1j * freqs[:, None] * DELAYS)
    Mm = invD[:, :, None] * np.eye(ND) - A
    bc = np.broadcast_to(b.astype(np.float64), (FB, ND, 2))
    X = np.linalg.solve(Mm, bc)
    H = np.einsum('ci,fio->fco', c.astype(complex), X)      # (FB, 2, 2)
    h = np.fft.irfft(H.transpose(1, 2, 0), n=L)             # (2, 2, L)
    # w_o = h[o,0] - j*h[o,1], zero-padded to N, in (128, 2048) row layout
    wt = np.zeros((2, 2, P1, M2), np.float32)
    for o in range(2):
        wr = np.zeros(N, np.float64); wr[:L] = h[o, 0]
        wi = np.zeros(N, np.float64); wi[:L] = -h[o, 1]
        wt[o, 0] = wr.reshape(P1, M2).astype(np.float32)
        wt[o, 1] = wi.reshape(P1, M2).astype(np.float32)
    return wt


# ---------------- device-side constants ----------------
def _consts():
    F = np.exp(-2j * np.pi * np.outer(np.arange(P1), np.arange(P1)) / P1)
    F16 = np.exp(-2j * np.pi * np.outer(np.arange(P3), np.arange(P3)) / P3)
    B = np.zeros((128, 128), complex)
    for k in range(8):
        B[k*16:(k+1)*16, k*16:(k+1)*16] = F16
    fmat = np.stack([F.real, F.imag, -F.imag]).astype(np.float32)
    bmat = np.stack([B.real, B.imag, -B.imag]).astype(np.float32)
    ident = np.eye(128, dtype=np.float32)
    # t1[k1, m] = W_N^{k1*m}
    t1c = np.exp(-2j * np.pi * np.outer(np.arange(P1), np.arange(M2)) / N)
    # t2[k2, k1*16+n3] = W_{2048}^{k2*n3}
    n3v = np.tile(np.arange(P3), P2)
    t2c = np.exp(-2j * np.pi * np.outer(np.arange(P2), n3v) / M2)
    # it1[n2, k1*16+n3] = W_N^{k1*(n2*16+n3)}  (conj applied in cmul)
    k1v = np.repeat(np.arange(P1), P3)[None, :]
    n3b = np.tile(np.arange(P3), P2)[None, :]
    n2v = np.arange(P2)[:, None]
    it1c = np.exp(-2j * np.pi * (k1v * (n2v * P3 + n3b)) / N)
    t1 = np.stack([t1c.real, t1c.imag]).astype(np.float32)
    t2 = np.stack([t2c.real, t2c.imag]).astype(np.float32)
    it1 = np.stack([it1c.real, it1c.imag]).astype(np.float32)
    return fmat, bmat, ident, t1, t2, it1


# ---------------- bass program ----------------
_PROG = None


def _mmdt(ap):
    return ap


def _build_program():
    import concourse.bass as bass
    import concourse.tile as tile
    from concourse import bacc, mybir

    f32 = mybir.dt.float32
    fmm = mybir.dt.float32r if USE_F32R else f32
    nc = bacc.Bacc("TRN2", target_bir_lowering=False, debug=False,
                   enable_asserts=False, num_devices=NCORES)

    xp = nc.dram_tensor("xp", [2, 2, T], f32, kind="ExternalInput").ap()
    wt = nc.dram_tensor("wt", [2, 2, P1, M2], f32, kind="ExternalInput").ap()
    fmat_d = nc.dram_tensor("fmat", [3, 128, 128], f32, kind="ExternalInput").ap()
    bmat_d = nc.dram_tensor("bmat", [3, 128, 128], f32, kind="ExternalInput").ap()
    id_d = nc.dram_tensor("ident", [128, 128], f32, kind="ExternalInput").ap()
    t1_d = nc.dram_tensor("t1", [2, 128, M2], f32, kind="ExternalInput").ap()
    t2_d = nc.dram_tensor("t2", [2, 128, M2], f32, kind="ExternalInput").ap()
    it1_d = nc.dram_tensor("it1", [2, 128, M2], f32, kind="ExternalInput").ap()
    yp = nc.dram_tensor("yp", [2, 2, T], f32, kind="ExternalOutput").ap()

    NCH = 4           # 512-wide chunks per 2048
    CW = 512

    from contextlib import ExitStack
    with tile.TileContext(nc) as tc, ExitStack() as ctx:
        cpool = ctx.enter_context(tc.tile_pool(name="consts", bufs=1))
        wpool = ctx.enter_context(tc.tile_pool(name="wspec", bufs=1))
        work = ctx.enter_context(tc.tile_pool(name="work", bufs=1))
        psA = ctx.enter_context(tc.tile_pool(name="psA", bufs=2, space="PSUM"))
        psT = ctx.enter_context(tc.tile_pool(name="psT", bufs=2, space="PSUM"))
        psTc = ctx.enter_context(tc.tile_pool(name="psTc", bufs=1, space="PSUM"))

        # constant tiles
        fr = cpool.tile([128, 128], f32, tag="fr"); nc.sync.dma_start(fr[:], fmat_d[0])
        fi = cpool.tile([128, 128], f32, tag="fi"); nc.sync.dma_start(fi[:], fmat_d[1])
        nfi = cpool.tile([128, 128], f32, tag="nfi"); nc.sync.dma_start(nfi[:], fmat_d[2])
        br = cpool.tile([128, 128], f32, tag="br"); nc.sync.dma_start(br[:], bmat_d[0])
        bi = cpool.tile([128, 128], f32, tag="bi"); nc.sync.dma_start(bi[:], bmat_d[1])
        nbi = cpool.tile([128, 128], f32, tag="nbi"); nc.sync.dma_start(nbi[:], bmat_d[2])
        idt = cpool.tile([128, 128], f32, tag="idt"); nc.sync.dma_start(idt[:], id_d[:, :])
        t1r = cpool.tile([128, M2], f32, tag="t1r"); nc.sync.dma_start(t1r[:], t1_d[0])
        t1i = cpool.tile([128, M2], f32, tag="t1i"); nc.sync.dma_start(t1i[:], t1_d[1])
        t2r = cpool.tile([128, M2], f32, tag="t2r"); nc.sync.dma_start(t2r[:], t2_d[0])
        t2i = cpool.tile([128, M2], f32, tag="t2i"); nc.sync.dma_start(t2i[:], t2_d[1])
        it1r = cpool.tile([128, M2], f32, tag="it1r"); nc.sync.dma_start(it1r[:], it1_d[0])
        it1i = cpool.tile([128, M2], f32, tag="it1i"); nc.sync.dma_start(it1i[:], it1_d[1])

        wtiles = [[wpool.tile([128, M2], f32, tag=f"w{o}{p}", name=f"w{o}{p}")
                   for p in range(2)] for o in range(2)]

        # device-rounded f32r copies of the DFT matrices (for compute-fed stages)
        fr_r = cpool.tile([128, 128], fmm, tag="fr_r"); nc.scalar.copy(fr_r[:], fr[:])
        fi_r = cpool.tile([128, 128], fmm, tag="fi_r"); nc.scalar.copy(fi_r[:], fi[:])
        nfi_r = cpool.tile([128, 128], fmm, tag="nfi_r"); nc.scalar.copy(nfi_r[:], nfi[:])
        br_r = cpool.tile([128, 128], fmm, tag="br_r"); nc.scalar.copy(br_r[:], br[:])
        bi_r = cpool.tile([128, 128], fmm, tag="bi_r"); nc.scalar.copy(bi_r[:], bi[:])
        nbi_r = cpool.tile([128, 128], fmm, tag="nbi_r"); nc.scalar.copy(nbi_r[:], nbi[:])

        def cmul_psum(dst_r, dst_i, pr, pi, trr, tri, conj, s0, w, eng1, eng2):
            """(dst_r+j dst_i)[:, s0:s0+w] = (pr + j pi) * (trr + j tri)[slice],
            psum inputs pr/pi (128, w). conj: multiply by conj of twiddle."""
            sc = work.tile([128, CW], f32, tag="cm1")
            sd = work.tile([128, CW], f32, tag="cm2")
            a = (s0, s0 + w)
            # psum-reading muls must be DVE; SBUF-only combines go to GpSimd
            eng1.tensor_mul(sc[:, :w], pr[:, :w], trr[:, a[0]:a[1]])
            eng1.tensor_mul(sd[:, :w], pi[:, :w], tri[:, a[0]:a[1]])
            if conj:
                eng2.tensor_add(dst_r[:, a[0]:a[1]], sc[:, :w], sd[:, :w])
            else:
                eng2.tensor_sub(dst_r[:, a[0]:a[1]], sc[:, :w], sd[:, :w])
            sc2 = work.tile([128, CW], f32, tag="cm3")
            sd2 = work.tile([128, CW], f32, tag="cm4")
            eng1.tensor_mul(sc2[:, :w], pr[:, :w], tri[:, a[0]:a[1]])
            eng1.tensor_mul(sd2[:, :w], pi[:, :w], trr[:, a[0]:a[1]])
            if conj:
                eng2.tensor_sub(dst_i[:, a[0]:a[1]], sd2[:, :w], sc2[:, :w])
            else:
                eng2.tensor_add(dst_i[:, a[0]:a[1]], sc2[:, :w], sd2[:, :w])

        def stage_mm(dst, rhs_r, rhs_i, mr, mi, nmi, conj, tw, post):
            """One FFT stage: out = (conj?)(F) @ (rhs_r + j rhs_i), then
            per-chunk post-processing. tw = (twr, twi, conj_tw) or None.
            dst = (dr, di) SBUF tiles; post='copy'/'cmul'/'real_scale'."""
            dr, di = dst
            for ch in range(NCH):
                s0 = ch * CW
                sl = (slice(None), slice(s0, s0 + CW))
                prr = psA.tile([128, CW], f32, tag="pr")
                pii = psA.tile([128, CW], f32, tag="pi")
                if not conj:
                    nc.tensor.matmul(prr[:], _mmdt(mr[:]), _mmdt(rhs_r[sl]), start=True, stop=False)
                    nc.tensor.matmul(prr[:], _mmdt(nmi[:]), _mmdt(rhs_i[sl]), start=False, stop=True)
                    nc.tensor.matmul(pii[:], _mmdt(mi[:]), _mmdt(rhs_r[sl]), start=True, stop=False)
                    nc.tensor.matmul(pii[:], _mmdt(mr[:]), _mmdt(rhs_i[sl]), start=False, stop=True)
                else:
                    nc.tensor.matmul(prr[:], _mmdt(mr[:]), _mmdt(rhs_r[sl]), start=True, stop=False)
                    nc.tensor.matmul(prr[:], _mmdt(mi[:]), _mmdt(rhs_i[sl]), start=False, stop=True)
                    nc.tensor.matmul(pii[:], _mmdt(mr[:]), _mmdt(rhs_i[sl]), start=True, stop=False)
                    nc.tensor.matmul(pii[:], _mmdt(nmi[:]), _mmdt(rhs_r[sl]), start=False, stop=True)
                if post == "cmul":
                    twr, twi, ctw = tw
                    cmul_psum(dr, di, prr, pii, twr, twi, ctw, s0, CW,
                              nc.vector, nc.vector)
                elif post == "copy":
                    nc.scalar.copy(dr[sl], prr[:])
                    nc.scalar.copy(di[sl], pii[:])

        def stage_mm_real(dst_r, rhs_r, rhs_i, mr, mi, scale):
            """Last inverse stage: real part only: Fr@vr + Fi@vi, scaled."""
            for ch in range(NCH):
                s0 = ch * CW
                sl = (slice(None), slice(s0, s0 + CW))
                prr = psA.tile([128, CW], f32, tag="pr")
                nc.tensor.matmul(prr[:], _mmdt(mr[:]), _mmdt(rhs_r[sl]), start=True, stop=False)
                nc.tensor.matmul(prr[:], _mmdt(mi[:]), _mmdt(rhs_i[sl]), start=False, stop=True)
                nc.scalar.mul(dst_r[sl], prr[:], scale)

        def t_slices_strided(src_r, src_i, dst_r, dst_i):
            """T1/T1': (p; a*16+n3) -> (a; p*16+n3): 16 strided slice transposes
            per plane, batched 4 slices -> one (128,512) psum, 3D copy out."""
            for pl, (s, d) in enumerate(((src_r, dst_r), (src_i, dst_i))):
                for grp in range(4):
                    pt = psT.tile([128, CW], f32, tag="pt")
                    for q in range(4):
                        n3 = grp * 4 + q
                        nc.tensor.transpose(pt[:, q*128:(q+1)*128],
                                            s[:, n3:M2:16], idt[:])
                    # psum free = (q, a): copy to dst[:, a*16 + (grp*4+q)]
                    psrc = pt[:].rearrange("p (q a) -> p a q", q=4)
                    ddst = d[:].rearrange("p (a b) -> p a b", b=16)[:, :, grp*4:grp*4+4]
                    nc.scalar.copy(ddst, psrc)

        def t_slices_contig(src_r, src_i, dst_r, dst_i, tw=None):
            """T2/T2': 16 contiguous 128-wide transposes; batched by 4 into
            (128,512) psum; copy (or conj-twiddle-cmul) to same free offsets."""
            for grp in range(4):
                ptr = psTc.tile([128, CW], f32, tag="ptr")
                pti = psTc.tile([128, CW], f32, tag="pti")
                for q in range(4):
                    g = grp * 4 + q
                    nc.tensor.transpose(ptr[:, q*128:(q+1)*128],
                                        src_r[:, g*128:(g+1)*128], idt[:])
                    nc.tensor.transpose(pti[:, q*128:(q+1)*128],
                                        src_i[:, g*128:(g+1)*128], idt[:])
                s0 = grp * CW
                if tw is None:
                    nc.scalar.copy(dst_r[:, s0:s0+CW], ptr[:])
                    nc.scalar.copy(dst_i[:, s0:s0+CW], pti[:])
                else:
                    twr, twi, ctw = tw
                    cmul_psum(dst_r, dst_i, ptr, pti, twr, twi, ctw, s0, CW,
                              nc.vector, nc.vector)

        def fwd_fft(in_r, in_i, zr, zi, ar, ai, br_t, bi_t):
            stage_mm((ar, ai), in_r, in_i, fr, fi, nfi, False,
                     (t1r, t1i, False), "cmul")
            t_slices_strided(ar, ai, br_t, bi_t)
            stage_mm((ar, ai), br_t, bi_t, fr_r, fi_r, nfi_r, False,
                     (t2r, t2i, False), "cmul")
            t_slices_contig(ar, ai, br_t, bi_t)
            stage_mm((zr, zi), br_t, bi_t, br_r, bi_r, nbi_r, False, None, "copy")

        def inv_fft(pr_t, pi_t, out_r, ar, ai, br_t, bi_t):
            stage_mm((ar, ai), pr_t, pi_t, br_r, bi_r, nbi_r, True, None, "copy")
            t_slices_contig(ar, ai, br_t, bi_t, tw=(t2r, t2i, True))
            stage_mm((ar, ai), br_t, bi_t, fr_r, fi_r, nfi_r, True,
                     (it1r, it1i, True), "cmul")
            t_slices_strided(ar, ai, br_t, bi_t)
            stage_mm_real(out_r, br_t, bi_t, fr_r, fi_r, 1.0 / N)

        def load_block(in_r, in_i, b, blk):
            for pl, t_ in ((0, in_r), (1, in_i)):
                src = xp[b, pl]
                dma = nc.sync if pl == 0 else nc.scalar
                if blk == 0:
                    nc.vector.memset(t_[0:VROW+1, :], 0.0)
                    dma.dma_start(t_[VROW:VROW+1, VCOL:M2], src[0:ROW_TAIL].rearrange('(a b) -> a b', a=1))
                    dma.dma_start(
                        t_[VROW+1:128, :],
                        src[ROW_TAIL:HOP].rearrange("(r m) -> r m", m=M2))
                elif blk == 1:
                    s0 = HOP - (L - 1)
                    dma.dma_start(
                        t_[:, :], src[s0:s0+N].rearrange("(r m) -> r m", m=M2))
                else:
                    s0 = 2 * HOP - (L - 1)
                    nfull = (T - s0) // M2            # 88 full rows
                    rem = (T - s0) - nfull * M2       # 1085
                    nc.vector.memset(t_[64:128, :], 0.0)  # 32-aligned start; DMA overwrites 64..88
                    dma.dma_start(
                        t_[0:nfull, :],
                        src[s0:s0+nfull*M2].rearrange("(r m) -> r m", m=M2))
                    dma.dma_start(t_[nfull:nfull+1, 0:rem], src[s0+nfull*M2:T].rearrange('(a b) -> a b', a=1))

        def store_block(out_r, b, o, blk):
            dst = yp[b, o]
            base = blk * HOP
            nc.scalar.dma_start(dst[base:base+ROW_TAIL].rearrange('(a b) -> a b', a=1), out_r[VROW:VROW+1, VCOL:M2])
            if blk < 2:
                nc.scalar.dma_start(
                    dst[base+ROW_TAIL:base+HOP].rearrange("(r m) -> r m", m=M2),
                    out_r[VROW+1:128, :])
            else:
                nrem = T - base - ROW_TAIL            # 91197
                nfull = nrem // M2                    # 44
                rem = nrem - nfull * M2               # 1085
                nc.scalar.dma_start(
                    dst[base+ROW_TAIL:base+ROW_TAIL+nfull*M2]
                        .rearrange("(r m) -> r m", m=M2),
                    out_r[VROW+1:VROW+1+nfull, :])
                nc.scalar.dma_start(dst[T-rem:T].rearrange('(a b) -> a b', a=1), out_r[VROW+1+nfull:VROW+2+nfull, 0:rem])

        def spectral_cmul(dr, di, zr_, zi_, wr_, wi_):
            for ch in range(4):
                s0 = ch * CW
                sl = (slice(None), slice(s0, s0 + CW))
                sc = work.tile([128, CW], f32, tag="sm1")
                sd = work.tile([128, CW], f32, tag="sm2")
                nc.vector.tensor_mul(sc[:, :], zr_[sl], wr_[sl])
                nc.gpsimd.tensor_mul(sd[:, :], zi_[sl], wi_[sl])
                nc.vector.tensor_sub(dr[sl], sc[:, :], sd[:, :])
                nc.vector.tensor_mul(sc[:, :], zr_[sl], wi_[sl])
                nc.gpsimd.tensor_mul(sd[:, :], zi_[sl], wr_[sl])
                nc.gpsimd.tensor_add(di[sl], sc[:, :], sd[:, :])

        # ---- program ----
        def wtile_pair():
            ar = work.tile([128, M2], f32, tag="ar")
            ai = work.tile([128, M2], f32, tag="ai")
            bt_r = work.tile([128, M2], fmm, tag="btr", bufs=2)
            bt_i = work.tile([128, M2], fmm, tag="bti", bufs=2)
            return ar, ai, bt_r, bt_i

        # W spectra (2 forward FFTs of host-padded w)
        for o in range(2):
            ar, ai, bt_r, bt_i = wtile_pair()
            in_r = work.tile([128, M2], f32, tag="inr")
            in_i = work.tile([128, M2], f32, tag="ini")
            nc.sync.dma_start(in_r[:], wt[o, 0])
            nc.scalar.dma_start(in_i[:], wt[o, 1])
            fwd_fft(in_r, in_i, wtiles[o][0], wtiles[o][1], ar, ai, bt_r, bt_i)

        for b in range(2):
            for blk in range(NBLK):
                zr = work.tile([128, M2], f32, tag="zr")
                zi = work.tile([128, M2], f32, tag="zi")
                ar, ai, bt_r, bt_i = wtile_pair()
                in_r = work.tile([128, M2], f32, tag="inr")
                in_i = work.tile([128, M2], f32, tag="ini")
                load_block(in_r, in_i, b, blk)
                fwd_fft(in_r, in_i, zr, zi, ar, ai, bt_r, bt_i)
                for o in range(2):
                    pr_t = work.tile([128, M2], fmm, tag="pr_s")
                    pi_t = work.tile([128, M2], fmm, tag="pi_s")
                    spectral_cmul(pr_t, pi_t, zr, zi,
                                  wtiles[o][0], wtiles[o][1])
                    ar2, ai2, bt_r2, bt_i2 = wtile_pair()
                    out_r = work.tile([128, M2], f32, tag="outr")
                    inv_fft(pr_t, pi_t, out_r, ar2, ai2, bt_r2, bt_i2)
                    store_block(out_r, b, o, blk)

    nc.compile()
    return nc


def _get_prog():
    global _PROG
    if _PROG is None:
        _PROG = _build_program()
    return _PROG


# ---------------- public entry ----------------
def kernel(x, b, c, U_raw, gamma_raw):
    from concourse import bass_utils

    x = np.ascontiguousarray(np.asarray(x, np.float32))
    wt = _host_ir(np.asarray(b, np.float32), np.asarray(c, np.float32),
                  np.asarray(U_raw, np.float32), np.asarray(gamma_raw, np.float32))
    fmat, bmat, ident, t1, t2, it1 = _consts()
    nc = _get_prog()

    in_maps = []
    for core in range(NCORES):
        in_maps.append({
            "xp": x[2*core:2*core+2],
            "wt": wt, "fmat": fmat, "bmat": bmat, "ident": ident,
            "t1": t1, "t2": t2, "it1": it1,
        })
    res = bass_utils.run_bass_kernel_spmd(nc, in_maps, core_ids=list(range(NCORES)))
    y = np.empty((16, 2, T), np.float32)
    for core in range(NCORES):
        y[2*core:2*core+2] = res.results[core]["yp"]
    return y
